# revision 3
# baseline (speedup 1.0000x reference)
"""Expert-parallel SwiGLU MLP for 8 Trainium2 NeuronCores — v7.

Structure (trace-driven, see v3/v4 notes):
  - "Ladder" startup: x row-chunk DMAs (2 HWDGE queues), PE transposes, and
    quarter-granular phase-1 units are interleaved in PROGRAM ORDER, so the
    PE FIFO reaches the first matmul as soon as x-quarter 0 is transposed
    (~19 us) instead of after the last transpose (~104 us in v3).
  - Phase-1 units are (fc, gq) quarter chains: pg/pu [128,512] psum, 32 MMs,
    one g-quarter each; mid5 mapping unchanged (gp = gq*2+j).
  - USE_BF16: all-bf16 matmuls with every fp32->bf16 cast on DVE (no SWDGE
    cast DMAs — suspected P0 power trigger), plus a co-resident first mid
    panel (mq0) that zeroes the phase boundary.  Otherwise fp32r phase 1
    (v1/v3 style, sustains 2.4 GHz) without mq0 (no SBUF room).
"""

import numpy as np

import concourse.mybir as mybir
import concourse.tile as tile
from concourse import bacc
from concourse.bass_utils import run_bass_kernel_spmd
from concourse.masks import make_identity

E, G, D, F = 8, 2048, 2048, 5632
P = 128
DO = D // P      # 16 d-chunks
FC = F // P      # 44 f-chunks
GO = G // P      # 16 g-chunks
FA = 12          # fo split for phase-2 w2 tiles: [0:FA) co-residable

USE_BF16 = True

F32 = mybir.dt.float32
F32R = mybir.dt.float32r
BF16 = mybir.dt.bfloat16
WDT = BF16 if USE_BF16 else F32R


def build_nc():
    nc = bacc.Bacc("TRN2", target_bir_lowering=False)
    x = nc.dram_tensor("x", [G, D], F32, kind="ExternalInput").ap()
    w1 = nc.dram_tensor("w1", [D, F], F32, kind="ExternalInput").ap()
    w2 = nc.dram_tensor("w2", [F, D], F32, kind="ExternalInput").ap()
    w3 = nc.dram_tensor("w3", [D, F], F32, kind="ExternalInput").ap()
    out = nc.dram_tensor("out", [G, D], F32, kind="ExternalOutput").ap()

    w1r = w1.rearrange("(do p) f -> p do f", p=P)
    w3r = w3.rearrange("(do p) f -> p do f", p=P)
    w2r = w2.rearrange("(fo p) d -> p fo d", p=P)

    with tile.TileContext(nc) as tc:
        dram = tc.alloc_tile_pool(name="dram", bufs=1, space="DRAM")
        # mid5[p, gp, fo, g'] = silu/up product for f = fo*128+p, g = gp*256+g'
        mid5 = dram.tile([P, 8, FC, 256], BF16)

        if USE_BF16:
            wsp = tc.alloc_tile_pool(name="wsp", bufs=3)  # fp32 staging
            w2sap = tc.alloc_tile_pool(name="w2sap", bufs=1)
        wp = tc.alloc_tile_pool(name="wp", bufs=4)
        mp = tc.alloc_tile_pool(name="mp", bufs=3)
        xtp = tc.alloc_tile_pool(name="xtp", bufs=1)
        xTq = [
            xtp.tile([P, DO, 512], WDT, tag=f"xT{qr}", name=f"xT{qr}")
            for qr in range(4)
        ]
        w2pa = tc.alloc_tile_pool(name="w2pa", bufs=2, side="right")

        p0 = tc.alloc_tile_pool(name="p0", bufs=5)
        idp = tc.alloc_tile_pool(name="idp", bufs=1)
        p0ps = tc.alloc_tile_pool(name="p0ps", bufs=4, space="PSUM")
        ps1g = tc.alloc_tile_pool(name="ps1g", bufs=2, space="PSUM")
        ps1u = tc.alloc_tile_pool(name="ps1u", bufs=2, space="PSUM")
        ident = idp.tile([P, P], F32)
        make_identity(nc, ident)

        def dma_x(go):
            for h in range(2):
                xsg = p0.tile([P, 1024], F32, tag="xs", name=f"xs{go}_{h}")
                eng = nc.sync if h == 0 else nc.scalar
                eng.dma_start(
                    xsg, x[go * P : (go + 1) * P, h * 1024 : (h + 1) * 1024]
                )
                yield xsg

        xs_tiles = {}

        def emit_x_dma(go):
            xs_tiles[go] = list(dma_x(go))

        def emit_transpose(go):
            qr, goff = go // 4, (go % 4) * P
            for h in range(2):
                xsg = xs_tiles[go][h]
                for q in range(2):
                    tp = p0ps.tile([P, 4, P], F32, tag="tp")
                    for j in range(4):
                        d0 = (q * 4 + j) * P
                        nc.tensor.transpose(tp[:, j], xsg[:, d0 : d0 + P], ident)
                    dbase = h * 8 + q * 4
                    nc.vector.tensor_copy(
                        xTq[qr][:, dbase : dbase + 4, goff : goff + P], tp
                    )

        w1ts, w3ts = {}, {}

        def load_w(fc):
            if USE_BF16:
                w1s = wsp.tile([P, DO, P], F32, tag="w1s", name=f"w1s{fc}")
                nc.gpsimd.dma_start(w1s, w1r[:, :, fc * P : (fc + 1) * P])
                w1t = wp.tile([P, DO, P], BF16, tag="w1", name=f"w1t{fc}")
                nc.vector.tensor_copy(w1t, w1s)
                w3s = wsp.tile([P, DO, P], F32, tag="w3s", name=f"w3s{fc}")
                nc.gpsimd.dma_start(w3s, w3r[:, :, fc * P : (fc + 1) * P])
                w3t = wp.tile([P, DO, P], BF16, tag="w3", name=f"w3t{fc}")
                nc.vector.tensor_copy(w3t, w3s)
            else:
                w1t = wp.tile([P, DO, P], F32R, tag="w1", name=f"w1t{fc}")
                nc.gpsimd.dma_start(w1t, w1r[:, :, fc * P : (fc + 1) * P])
                w3t = wp.tile([P, DO, P], F32R, tag="w3", name=f"w3t{fc}")
                nc.gpsimd.dma_start(w3t, w3r[:, :, fc * P : (fc + 1) * P])
            w1ts[fc], w3ts[fc] = w1t, w3t

        def fc_unit(fc, gq):
            """one quarter-chain: mid[:, 2gq:2gq+2, fc] from x-quarter gq"""
            w1t, w3t = w1ts[fc], w3ts[fc]
            xt = xTq[gq]
            pg = ps1g.tile([P, 512], F32, tag="pg")
            pu = ps1u.tile([P, 512], F32, tag="pu")
            for d in range(DO):
                nc.tensor.matmul(
                    pg, w1t[:, d], xt[:, d], start=(d == 0), stop=(d == DO - 1)
                )
            for d in range(DO):
                nc.tensor.matmul(
                    pu, w3t[:, d], xt[:, d], start=(d == 0), stop=(d == DO - 1)
                )
            mo = mp.tile([P, 2, 256], BF16, tag="mo")
            mof = mo.rearrange("p j g -> p (j g)")
            nc.scalar.activation(mof, pg, mybir.ActivationFunctionType.Silu)
            nc.vector.tensor_mul(mof, mof, pu)
            for j in range(2):
                nc.scalar.dma_start(mid5[:, gq * 2 + j, fc], mo[:, j])

        # ---- ladder: x quarters + transposes + early fc units interleave
        for go in range(8):
            emit_x_dma(go)
        load_w(0)
        load_w(1)
        for go in range(4):
            emit_transpose(go)
        for go in range(8, 12):
            emit_x_dma(go)
        fc_unit(0, 0)
        for go in range(4, 8):
            emit_transpose(go)
        fc_unit(1, 0)
        for go in range(12, 16):
            emit_x_dma(go)
        fc_unit(0, 1)
        for go in range(8, 12):
            emit_transpose(go)
        fc_unit(1, 1)
        # dq=0's fo<FA w2 slice loads during phase 1
        w2qa0 = w2pa.tile([P, FA, 512], BF16, tag="w2qa")
        if USE_BF16:
            for k in range(2):
                w2sa0 = w2sap.tile(
                    [P, 6, 512], F32, tag="w2sa", name=f"w2sa0_{k}"
                )
                nc.gpsimd.dma_start(
                    w2sa0, w2r[:, 6 * k : 6 * k + 6, 0:512]
                )
                nc.vector.tensor_copy(
                    w2qa0[:, 6 * k : 6 * k + 6, :], w2sa0
                )
        else:
            for k in range(4):
                nc.gpsimd.dma_start(
                    w2qa0[:, 3 * k : 3 * k + 3, :],
                    w2r[:, 3 * k : 3 * k + 3, 0:512],
                )
        for go in range(12, 16):
            emit_transpose(go)
        load_w(2)
        for gq in (2, 3):
            fc_unit(0, gq)
            fc_unit(1, gq)
        load_w(3)
        for fc in range(2, FC):
            if fc + 2 < FC:
                load_w(fc + 2)
            for gq in range(4):
                fc_unit(fc, gq)
        ps1u.release()
        ps1g.release()
        p0ps.release()
        idp.release()
        p0.release()
        xtp.release()
        mp.release()
        wp.release()
        if USE_BF16:
            w2sap.release()
            wsp.release()

        # ---- phase 2: out[g, d] = midT.T @ w2 (bf16 x bf16, fp32 psum)
        w2pb = tc.alloc_tile_pool(name="w2pb", bufs=2, side="right")
        if USE_BF16:
            w2sp = tc.alloc_tile_pool(name="w2sp", bufs=2, side="right")
        mqp = tc.alloc_tile_pool(name="mqp", bufs=3, side="right")
        op = tc.alloc_tile_pool(name="op", bufs=6, side="right")
        ps2 = tc.alloc_tile_pool(name="ps2", bufs=3, space="PSUM")
        bbounds = [12, 16, 20, 24, 28, 32, 36, 40, 44]
        mbounds = [0, 6, 12, 18, 24, 29, 34, 39, 44]
        for dq in range(4):
            dsl = slice(dq * 512, (dq + 1) * 512)
            if dq == 0:
                w2qa = w2qa0
            else:
                w2qa = w2pa.tile([P, FA, 512], BF16, tag="w2qa")
                for k in range(4):
                    if USE_BF16:
                        w2s = w2sp.tile(
                            [P, 3, 512], F32, tag="w2s", name=f"w2sa{dq}_{k}"
                        )
                        nc.gpsimd.dma_start(w2s, w2r[:, 3 * k : 3 * k + 3, dsl])
                        nc.vector.tensor_copy(w2qa[:, 3 * k : 3 * k + 3, :], w2s)
                    else:
                        nc.gpsimd.dma_start(
                            w2qa[:, 3 * k : 3 * k + 3, :],
                            w2r[:, 3 * k : 3 * k + 3, dsl],
                        )
            w2qb = w2pb.tile([P, FC - FA, 512], BF16, tag="w2qb")
            for k in range(8):
                lo, hi = bbounds[k], bbounds[k + 1]
                if USE_BF16:
                    w2s = w2sp.tile(
                        [P, 4, 512], F32, tag="w2s", name=f"w2sb{dq}_{k}"
                    )
                    nc.gpsimd.dma_start(w2s, w2r[:, lo:hi, dsl])
                    nc.vector.tensor_copy(w2qb[:, lo - FA : hi - FA, :], w2s)
                else:
                    nc.gpsimd.dma_start(
                        w2qb[:, lo - FA : hi - FA, :], w2r[:, lo:hi, dsl]
                    )
            for gp in range(8):
                mq = mqp.tile([P, FC, 256], BF16, tag="mq")
                if dq == 0 and gp == 0:
                    for k in range(8):
                        lo, hi = mbounds[k], mbounds[k + 1]
                        eng = nc.sync if k % 2 == 0 else nc.scalar
                        eng.dma_start(mq[:, lo:hi, :], mid5[:, 0, lo:hi, :])
                else:
                    nc.sync.dma_start(mq[:, 0:22, :], mid5[:, gp, 0:22, :])
                    nc.scalar.dma_start(mq[:, 22:FC, :], mid5[:, gp, 22:FC, :])
                po = ps2.tile([P, 2, 512], F32, tag="po")
                for fo in range(FC):
                    w2m = w2qa[:, fo] if fo < FA else w2qb[:, fo - FA]
                    for gc in range(2):
                        nc.tensor.matmul(
                            po[:, gc],
                            mq[:, fo, gc * P : (gc + 1) * P],
                            w2m,
                            start=(fo == 0),
                            stop=(fo == FC - 1),
                        )
                for gc in range(2):
                    ot = op.tile([P, 512], F32, tag="ot")
                    nc.vector.tensor_copy(ot, po[:, gc])
                    g0 = (gp * 2 + gc) * P
                    nc.scalar.dma_start(out[g0 : g0 + P, dsl], ot)
        op.release()
        mqp.release()
        if USE_BF16:
            w2sp.release()
        w2pb.release()
        w2pa.release()
        ps2.release()
        dram.release()
    nc.compile()
    return nc


_NC_CACHE = None


def _get_nc():
    global _NC_CACHE
    if _NC_CACHE is None:
        _NC_CACHE = build_nc()
    return _NC_CACHE


def _in_maps(routed_in_egD, w1, w2, w3):
    x = np.ascontiguousarray(np.asarray(routed_in_egD, dtype=np.float32))
    w1 = np.ascontiguousarray(np.asarray(w1, dtype=np.float32))
    w2 = np.ascontiguousarray(np.asarray(w2, dtype=np.float32))
    w3 = np.ascontiguousarray(np.asarray(w3, dtype=np.float32))
    x_e = x.reshape(E, G, D)
    return [
        {"x": x_e[e], "w1": w1[e], "w2": w2[e], "w3": w3[e]} for e in range(E)
    ]


def kernel(routed_in_egD, w1, w2, w3):
    nc = _get_nc()
    in_maps = _in_maps(routed_in_egD, w1, w2, w3)
    try:
        res = run_bass_kernel_spmd(nc, in_maps, core_ids=list(range(E)))
    except Exception:
        res = run_bass_kernel_spmd(nc, in_maps, core_ids=list(range(E)))
    return np.concatenate([r["out"] for r in res.results], axis=0)


def run_traced(routed_in_egD, w1, w2, w3, **trace_kwargs):
    nc = _get_nc()
    res = run_bass_kernel_spmd(
        nc,
        _in_maps(routed_in_egD, w1, w2, w3),
        core_ids=list(range(E)),
        trace=True,
        **trace_kwargs,
    )
    out = np.concatenate([r["out"] for r in res.results], axis=0)
    return out, res
